# revision 11
# baseline (speedup 1.0000x reference)
"""Biased multi-head attention on 8 Trainium2 NeuronCores.

Sharding: batch x head-group. Core c handles batch b = c//4 and heads
4*(c%4) .. 4*(c%4)+3 (4 of 16 heads). Q/K/V projections are column-sharded
over the core's heads, scores/softmax/AV are fully local per head, and the
output projection is row-sharded (each core contributes a partial [D, L]
that the host sums per batch).

Device-side dataflow (per core):
  - x arrives pre-transposed (xT [D, L]); projections contract over D on
    the partition dim with no on-device transposes.
  - Q/K are produced transposed (qkT [c, l]); scores are computed
    transposed, S_T[k, q] = kT.T @ qT per head (contraction c=64).
  - attn_bias arrives pre-transposed per head (biasT [k, q]) to match S_T;
    the key-padding mask folds into the exp() activation's per-partition
    bias. Softmax skips max-subtraction and the +-20 clamp: scores are
    ~N(0,2) (exp never overflows) and padded keys sit at ~-1e4 (exp
    underflows to 0); the reference's clamp only binds for padded keys,
    whose post-softmax weight ~1e-11 is below fp32 noise.
  - V is produced in [l, c] layout with a ones column appended; the AV
    matmul (lhsT = [V | 1], rhs = exp(S_T)) yields O_T[c, q] and the
    softmax denominator Z[q] (row 64) in one accumulation group, already
    in the layout the out-projection wants.
  - 1/Z is broadcast from row 64 to 64 partitions by a 2KB DRAM bounce
    (partition-stride-0 DRAM reads are legal, SBUF ones are not), then
    O_T rows are scaled on DVE, biased on ACT, and staged to a DRAM OT
    buffer, which is re-read once as [128, 2, L] for the out-projection.

All heavy matmuls use float32r (fp32 rounded to 11-bit mantissa; single
pass at 1 cycle/row for free dim >= 256, vs strict fp32's two half-speed
LOW/HIGH passes). The DMA path rounds f32->f32r in flight, and ACT/DVE
round when writing f32r tiles.
"""

import os

import numpy as np

B, L, D, H = 2, 2048, 1024, 16
dh = D // H          # 64
NCORES = 8
HPC = 4              # heads per core
P = 128

_compiled = None     # built+compiled Bass module, cached per process
LAST_RESULT = None   # BassKernelResults of the most recent run (for profiling)


def _build():
    from contextlib import ExitStack

    import concourse.bass as bass
    import concourse.tile as tile
    from concourse import bacc, mybir
    from concourse.bass import ds, ts

    f32 = mybir.dt.float32
    f32r = mybir.dt.float32r
    Act = mybir.ActivationFunctionType

    nc = bacc.Bacc("TRN2", target_bir_lowering=False, debug=False,
                   num_devices=NCORES)

    xT_d = nc.dram_tensor("xT", [D, L], f32r, kind="ExternalInput").ap()
    wqkT_d = nc.dram_tensor("wqkT", [D, 512], f32r, kind="ExternalInput").ap()
    wvT_d = nc.dram_tensor("wvT", [D, 256], f32r, kind="ExternalInput").ap()
    bqk_d = nc.dram_tensor("bqk", [512], f32, kind="ExternalInput").ap()
    bv4_d = nc.dram_tensor("bv4", [64, HPC], f32, kind="ExternalInput").ap()
    maskT_d = nc.dram_tensor("maskT", [L], f32, kind="ExternalInput").ap()
    biasT_d = nc.dram_tensor("biasT", [HPC, L, L], f32, kind="ExternalInput").ap()
    woutT_d = nc.dram_tensor("woutT", [256, D], f32r, kind="ExternalInput").ap()
    bout_d = nc.dram_tensor("bout4", [D], f32, kind="ExternalInput").ap()
    outT_d = nc.dram_tensor("outT", [D, L], f32, kind="ExternalOutput").ap()

    with tile.TileContext(nc) as tc, ExitStack() as ctx:
        consts = ctx.enter_context(tc.tile_pool(name="consts", bufs=1))
        xp = ctx.enter_context(tc.tile_pool(name="xp", bufs=2))
        biasp = ctx.enter_context(tc.tile_pool(name="biasp", bufs=4))
        expp = ctx.enter_context(tc.tile_pool(name="expp", bufs=3))
        normp = ctx.enter_context(tc.tile_pool(name="normp", bufs=2))
        outp = ctx.enter_context(tc.tile_pool(name="outp", bufs=3))
        dramp = ctx.enter_context(tc.tile_pool(name="dramp", bufs=2,
                                               space="DRAM"))
        psS = ctx.enter_context(tc.tile_pool(name="psS", bufs=6, space="PSUM"))
        psAV = ctx.enter_context(tc.tile_pool(name="psAV", bufs=2, space="PSUM"))

        wqkT_sb = consts.tile([P, 8, 512], f32r, name="wqkT_sb", tag="wqkT_sb")
        nc.sync.dma_start(wqkT_sb, wqkT_d.rearrange("(o p) m -> p o m", p=P))
        wvT_sb = consts.tile([P, 8, 256], f32r, name="wvT_sb", tag="wvT_sb")
        nc.sync.dma_start(wvT_sb, wvT_d.rearrange("(o p) m -> p o m", p=P))
        woutT_sb = consts.tile([P, 2, D], f32r, name="woutT_sb", tag="woutT_sb")
        nc.sync.dma_start(woutT_sb, woutT_d.rearrange("(o p) m -> p o m", p=P))
        bqk_sb = consts.tile([P, 4], f32, name="bqk_sb", tag="bqk_sb")
        nc.sync.dma_start(bqk_sb, bqk_d.rearrange("(o p) -> p o", p=P))
        bv4_sb = consts.tile([64, HPC], f32, name="bv4_sb", tag="bv4_sb")
        nc.sync.dma_start(bv4_sb, bv4_d)
        maskT_sb = consts.tile([P, 16], f32, name="maskT_sb", tag="maskT_sb")
        nc.sync.dma_start(maskT_sb, maskT_d.rearrange("(o p) -> p o", p=P))
        bout_sb = consts.tile([P, 8], f32, name="bout_sb", tag="bout_sb")
        nc.sync.dma_start(bout_sb, bout_d.rearrange("(o p) -> p o", p=P))

        qkT_sb = consts.tile([P, 4, L], f32r, name="qkT_sb", tag="qkT_sb")
        V_sb = consts.tile([P, 16, HPC, 65], f32r, name="V_sb", tag="V_sb")
        OT_sb = consts.tile([P, 2, L], f32r, name="OT_sb", tag="OT_sb")
        OTd = dramp.tile([256, L], f32r, name="OTd", tag="OTd", bufs=1)

        ones_c = consts.tile([P, 1], f32, name="ones_c", tag="ones_c")
        nc.vector.memset(ones_c, 1.0)
        nc.vector.tensor_copy(
            V_sb[:, :, :, 64:65],
            ones_c[:, 0:1, None, None].to_broadcast((P, 16, HPC, 1)),
        )

        # ---- Phase 1: projections -------------------------------------
        # qkT[r, l] = (W_qk @ x.T)[r, l] (+bias, q rows pre-scaled by 1/8)
        for lt in range(4):
            xs = xp.tile([P, 8, 512], f32r, name="xs", tag="xs")
            nc.sync.dma_start(
                xs, xT_d[:, ts(lt, 512)].rearrange("(o p) l -> p o l", p=P))
            for rt in range(4):
                ps = psS.tile([P, 512], f32, name="ps_s", tag="ps_s")
                for dc in range(8):
                    nc.tensor.matmul(
                        ps,
                        lhsT=wqkT_sb[:, dc, ts(rt, P)],
                        rhs=xs[:, dc, :],
                        start=(dc == 0), stop=(dc == 7),
                    )
                nc.scalar.activation(
                    qkT_sb[:, rt, ts(lt, 512)], ps, Act.Identity,
                    bias=bqk_sb[:, rt:rt + 1],
                    scale=(0.125 if rt < 2 else 1.0),
                )
            # V[l, c] for the 4 heads (c-major per head); bv added post-Z
            for l4 in range(4):
                ltv = lt * 4 + l4
                psv = psS.tile([P, 512], f32, name="ps_s", tag="ps_s")[:, :256]
                for dc in range(8):
                    nc.tensor.matmul(
                        psv,
                        lhsT=xs[:, dc, ts(l4, P)],
                        rhs=wvT_sb[:, dc, :],
                        start=(dc == 0), stop=(dc == 7),
                    )
                nc.vector.tensor_copy(
                    V_sb[:, ltv, :, 0:64],
                    psv.rearrange("p (h c) -> p h c", c=64),
                )

        # ---- Phase 2: attention ---------------------------------------
        # Heads are processed in pairs: the even head's tensors sit on
        # partitions 0:64, the odd head's on 64:128, so the two scores
        # matmuls (K=64 each) occupy disjoint PE row groups and stream
        # concurrently through the array.
        for pair in range(2):
            ko, qo = 2 + pair, pair
            for qt in range(4):
                avs = [psAV.tile([65, 512], f32, name=f"av{hi}", tag="av")
                       for hi in range(2)]
                for kt in range(16):
                    btw = biasp.tile([P, 2, 512], f32, name="btw", tag="btw")
                    nc.sync.dma_start(
                        btw,
                        biasT_d[2 * pair:2 * pair + 2, ts(kt, P),
                                ts(qt, 512)].rearrange("h k q -> k h q"),
                    )
                    for hi in range(2):
                        h = 2 * pair + hi
                        cs = slice(64 * hi, 64 * hi + 64)
                        sps = psS.tile([P, 512], f32, name="ps_s", tag="ps_s")
                        nc.tensor.matmul(
                            sps,
                            lhsT=qkT_sb[cs, ko, ts(kt, P)],
                            rhs=qkT_sb[cs, qo, ts(qt, 512)],
                            start=True, stop=True,
                        )
                        nc.vector.tensor_add(btw[:, hi, :], sps, btw[:, hi, :])
                    ex2 = expp.tile([P, 2, 512], f32r, name="ex2", tag="ex2")
                    nc.scalar.activation(
                        ex2, btw, Act.Exp,
                        bias=maskT_sb[:, kt:kt + 1], scale=1.0,
                    )
                    for hi in range(2):
                        h = 2 * pair + hi
                        nc.tensor.matmul(
                            avs[hi],
                            lhsT=V_sb[:, kt, h, :],
                            rhs=ex2[:, hi, :],
                            start=(kt == 0), stop=(kt == 15),
                        )
                for hi in range(2):
                    h = 2 * pair + hi
                    av = avs[hi]
                    # normalize: Z broadcast via 2KB DRAM bounce, then
                    # approx reciprocal on all 64 lanes
                    zrow = normp.tile([65, 512], f32, name="zrow", tag="zrow")
                    nc.scalar.copy(zrow[64:65, :], av[64:65, :])
                    zscr = dramp.tile([1, 512], f32, name="zscr", tag="zscr")
                    nc.gpsimd.dma_start(zscr, zrow[64:65, :])
                    zb = normp.tile([64, 512], f32, name="zb", tag="zb")
                    nc.gpsimd.dma_start(
                        zb,
                        bass.AP(tensor=zscr.tensor, offset=zscr.offset,
                                ap=[[0, 64], [1, 512]]),
                    )
                    zr = normp.tile([64, 512], f32, name="zr", tag="zr")
                    nc.vector.reciprocal_approx_fast(zr, zb)
                    t1 = normp.tile([64, 512], f32, name="t1", tag="t1")
                    nc.vector.tensor_mul(t1, av[0:64, :], zr)
                    ot2 = normp.tile([64, 512], f32r, name="ot2", tag="ot2")
                    nc.scalar.activation(
                        ot2, t1, Act.Identity, bias=bv4_sb[:, h:h + 1],
                        scale=1.0,
                    )
                    nc.gpsimd.dma_start(OTd[ts(h, 64), ts(qt, 512)], ot2)

        # reload staged O_T as [128, 2, L] for the out-projection
        nc.gpsimd.dma_start(OT_sb, OTd[:].rearrange("(o p) l -> p o l", p=P))

        # ---- Phase 3: output projection -------------------------------
        for jt in range(8):
            for it in range(4):
                ps = psS.tile([P, 512], f32, name="ps_s", tag="ps_s")
                for cc in range(2):
                    nc.tensor.matmul(
                        ps,
                        lhsT=woutT_sb[:, cc, ts(jt, P)],
                        rhs=OT_sb[:, cc, ts(it, 512)],
                        start=(cc == 0), stop=(cc == 1),
                    )
                osb = outp.tile([P, 512], f32, name="osb", tag="osb")
                nc.scalar.activation(
                    osb, ps, Act.Identity,
                    bias=bout_sb[:, jt:jt + 1], scale=1.0,
                )
                nc.sync.dma_start(outT_d[ts(jt, P), ts(it, 512)], osb)

    nc.compile()
    return nc


def _prep_core_inputs(c, x, key_padding_mask, attn_bias, W_in, b_in, W_out,
                      b_out):
    b, hg = c // HPC, c % HPC
    hs = slice(256 * hg, 256 * hg + 256)
    f32 = np.float32
    wq, wk, wv = W_in[0:D][hs], W_in[D:2 * D][hs], W_in[2 * D:3 * D][hs]
    return {
        "xT": np.ascontiguousarray(x[b].T, dtype=f32),
        "wqkT": np.ascontiguousarray(np.concatenate([wq, wk], 0).T, dtype=f32),
        "wvT": np.ascontiguousarray(wv.T, dtype=f32),
        "bqk": np.concatenate([b_in[0:D][hs] / 8.0, b_in[D:2 * D][hs]]).astype(f32),
        "bv4": np.ascontiguousarray(
            b_in[2 * D:3 * D][hs].reshape(HPC, 64).T, dtype=f32),
        "maskT": (-10000.0 * key_padding_mask[b]).astype(f32),
        "biasT": np.ascontiguousarray(
            attn_bias[b, HPC * hg:HPC * hg + HPC].transpose(0, 2, 1), dtype=f32),
        "woutT": np.ascontiguousarray(W_out[:, hs].T, dtype=f32),
        "bout4": (b_out / float(HPC)).astype(f32),
    }


def kernel(x, key_padding_mask, attn_bias, W_in, b_in, W_out, b_out):
    global _compiled, LAST_RESULT
    from concourse.bass_utils import run_bass_kernel_spmd

    if _compiled is None:
        _compiled = _build()

    in_maps = [
        _prep_core_inputs(c, x, key_padding_mask, attn_bias, W_in, b_in,
                          W_out, b_out)
        for c in range(NCORES)
    ]
    res = run_bass_kernel_spmd(
        _compiled, in_maps, core_ids=list(range(NCORES)),
        trace_cores=(list(range(NCORES))
                     if os.environ.get("BASS_TRACE") == "1" else None),
    )
    LAST_RESULT = res

    out = np.empty((B, L, D), dtype=np.float32)
    for b in range(B):
        acc = res.results[b * HPC]["outT"].astype(np.float64)
        for g in range(1, HPC):
            acc = acc + res.results[b * HPC + g]["outT"]
        out[b] = acc.T.astype(np.float32)
    return out
